# revision 37
# baseline (speedup 1.0000x reference)
"""Trainium2 Bass kernel for nn_O3TensorProductWeighted.

Computes, for each sample e:
    h  = relu(relu(weight @ W0 + b0) @ W1 + b1)           # [64]
    w  = h @ W2 + b2                                      # [36864] (never materialized)
    out0 = PW0*(einsum(Wa,s1)*s2 + I3*einsum(Wd,dot12))
    out1 = PW1*I3*(einsum(Wb,s1) x v2 + einsum(Wc,v1)*s2)
    out  = concat(out0, out1)/SQRT_K ; out[:128] += bias

Strategy: reassociate each einsum against the (k,u)-joint contraction of the
per-sample Khatri-Rao product h (x) x, so everything becomes dense matmuls
over shared W2 chunk weights, with the per-sample products built on-chip.
8 chunks of 8 k-values each; one PE broadcast matmul per chunk replicates
the 8 h2 rows 16x, ACT applies the relu (with a 2^2 scale folded in), and
tensor_mul ops (Pool for fp8 A/B products, DVE 2x mode for bf16 C/D) build
the per-sample Khatri-Rao products.

The A (0e x 0e) and B (0e x 1o) paths run as fp8-e4m3 DoubleRow matmuls
(two 128-row tiles summed per instruction at 0.5 cycles/row): weights are
split into an fp8 main + same-scale fp8 residual (two DR instructions) so
only the fp8 product rounding contributes error. C/D paths stay bf16.

Scheduling: C/D matmuls trail their chunk by 2 so the PE never waits on
the big finCD stream at the head; fp8 residual DR work is deferred to the
second half of the loop; the b2-bias matmuls ride mid-loop; small DRAM
parameters are packed into combined tensors to amortize per-DMA setup;
weight streams are chunk-sliced and interleaved. Scale compensation
(alpha*beta) is folded into host-prescaled epilogue operands. The epilogue
runs in two sample-halves so DVE work overlaps the PE transposes and
output DMA. Pure data parallel over 8 cores, transposed layout (features
on partitions, samples on the free dim).
"""

import dataclasses
import sys

sys.path.insert(0, "/opt/trn_rl_repo")

from contextlib import ExitStack

import ml_dtypes
import numpy as np

import concourse.bacc as bacc
import concourse.bass as bass
import concourse.tile as tile
from concourse import mybir
from concourse.bass_utils import run_bass_kernel_spmd

BF16 = mybir.dt.bfloat16
FP8 = mybir.dt.float8e4
F32 = mybir.dt.float32
BF16_NP = ml_dtypes.bfloat16
FP8_NP = ml_dtypes.float8_e4m3

N_CORES = 8
N = 4096
E = N // N_CORES  # 512 samples per core

MUL0, MUL1 = 128, 64
N1 = MUL0 * MUL0          # 16384
N2 = MUL0 * MUL1          # 8192
N3 = MUL1 * MUL1          # 4096
I3 = float(1.0 / np.sqrt(3.0))
# PW0/SQRT_K == 1.0 and PW1*I3/SQRT_K == 1.0 exactly; only I3 remains on D.

KPC = 4                   # k-values per chunk
G = 64 // KPC             # 16 chunks
TPB = 128 // KPC          # 32 u-values per row-block
NSL = 4                   # chunk-slices per big stationary stream
NBA = 128 // TPB          # 4 A/B x-blocks per chunk
NJA = NBA // 2            # 2 DoubleRow pairs per chunk on A/B
NJC = 64 // TPB           # 2 row-blocks per chunk on C/D
CD_LAG0 = 2               # C0/C1 matmuls trail their chunk by this many slots
CD_LAG1 = 3               # C2/D matmuls trail by this many slots
ALPHA = 1024.0            # fp8 scale on A/B stationary weights
BETA = 4.0                # scale folded into the relu -> rides on products
AB_INV = 1.0 / (ALPHA * BETA)
POOL_CD = {5, 10}         # chunks whose C/D products build on Pool (A/B: all)


def _build_nc():
    nc = bacc.Bacc(None)

    dp = nc.declare_dram_parameter
    # packed startup params
    wTw0_d = dp("wTw0", [17, E + 64], BF16, isOutput=False)  # wT;1 | w0;b0
    wg1_d = dp("wg1", [64, G * 128], BF16, isOutput=False)
    m128_d = dp("m128", [128, G + 1 + 128], F32, isOutput=False)  # bg1|bcol|ident
    # bigA0: finA first-half blocks ; bigA1: rest | s1t | ba | bb
    bigA0_d = dp("bigA0", [128, (NBA // 2) * E], BF16, isOutput=False)
    bigA1_d = dp("bigA1", [128, (NBA // 2) * E + E + 128 + 64], BF16,
                 isOutput=False)
    # finCD halves: [C0,C1] and [C2,D] blocks (2*NJC each)
    finCD_d = [dp(f"finCD{h}", [128, 2 * NJC * E], BF16, isOutput=False)
               for h in range(2)]
    # m64: vsall (3E) | d2t (E) | bc (64) | bd (128)
    m64_d = dp("m64", [64, 3 * E + E + 64 + 128], BF16, isOutput=False)
    v2p_d = dp("v2p", [64, 3 * E], F32, isOutput=False)
    s2b_d = dp("s2b", [128, E], F32, isOutput=False)
    # A/B stationaries: [128, G, NJA(jp), 2(tile), W] fp8 main + residual
    wa8m_d = dp("wa8m", [128, G * NJA * 2 * 128], FP8, isOutput=False)
    wa8r_d = dp("wa8r", [128, G * NJA * 2 * 128], FP8, isOutput=False)
    wb8m_d = dp("wb8m", [128, G * NJA * 2 * 64], FP8, isOutput=False)
    wb8r_d = dp("wb8r", [128, G * NJA * 2 * 64], FP8, isOutput=False)
    # C/D stationaries: [128, G, NJC(j), W] bf16
    wc_d = dp("wc", [128, G * NJC * 64], BF16, isOutput=False)
    wd_d = dp("wd", [128, G * NJC * 128], BF16, isOutput=False)

    outp_d = dp("outp", [E, 320], F32, isOutput=True)

    with tile.TileContext(nc) as tc, ExitStack() as ctx:
        const = ctx.enter_context(tc.tile_pool(name="const", bufs=1))
        work = ctx.enter_context(tc.tile_pool(name="work", bufs=1))
        bct_pool = ctx.enter_context(tc.tile_pool(name="bct", bufs=16))
        # pt8 tiles must survive ~4 chunks (residual DRs are deferred)
        pt8_pool = ctx.enter_context(tc.tile_pool(name="pt8", bufs=17))
        ptc_pool = ctx.enter_context(tc.tile_pool(name="ptc", bufs=16))
        out_pool = ctx.enter_context(tc.tile_pool(name="outs", bufs=4))
        ps_acc = ctx.enter_context(tc.tile_pool(name="ps_acc", bufs=1, space="PSUM"))
        ps_rot = ctx.enter_context(tc.tile_pool(name="ps_rot", bufs=2, space="PSUM"))

        def load(dparam, engine=None):
            t = const.tile(dparam.shape, dparam.dtype, name=f"t_{dparam.name}")
            (engine or nc.sync).dma_start(t[:], dparam[:])
            return t

        def sliced_tiles(dparam, n):
            w = dparam.shape[1] // n
            return [
                const.tile([dparam.shape[0], w], dparam.dtype,
                           name=f"t_{dparam.name}_{i}")
                for i in range(n)
            ], w

        # startup-critical on Pool SWDGE; streams on SP HWDGE by first use
        wTw0_t = load(wTw0_d, nc.gpsimd)

        m128_t = load(m128_d)
        wg1_t = load(wg1_d)
        bigA0_t = load(bigA0_d)
        wa8m_ts, wa8m_w = sliced_tiles(wa8m_d, NSL)
        wb8m_ts, wb8m_w = sliced_tiles(wb8m_d, NSL)
        wc_ts, wc_w = sliced_tiles(wc_d, NSL)
        wd_ts, wd_w = sliced_tiles(wd_d, NSL)
        wa8r_ts, _ = sliced_tiles(wa8r_d, NSL)
        wb8r_ts, _ = sliced_tiles(wb8r_d, NSL)

        def stream(i):
            for ts, d, w in ((wa8m_ts, wa8m_d, wa8m_w), (wb8m_ts, wb8m_d, wb8m_w),
                             (wc_ts, wc_d, wc_w), (wd_ts, wd_d, wd_w),
                             (wa8r_ts, wa8r_d, wa8m_w), (wb8r_ts, wb8r_d, wb8m_w)):
                nc.sync.dma_start(ts[i][:], d[:, i * w: (i + 1) * w])

        nc.sync.dma_start(wa8m_ts[0][:], wa8m_d[:, 0:wa8m_w])
        m64_t = load(m64_d)
        bigA1_t = load(bigA1_d)
        nc.sync.dma_start(wb8m_ts[0][:], wb8m_d[:, 0:wb8m_w])
        nc.sync.dma_start(wc_ts[0][:], wc_d[:, 0:wc_w])
        nc.sync.dma_start(wd_ts[0][:], wd_d[:, 0:wd_w])
        finCD_t = [load(d) for d in finCD_d]
        nc.sync.dma_start(wa8r_ts[0][:], wa8r_d[:, 0:wa8m_w])
        nc.sync.dma_start(wb8r_ts[0][:], wb8r_d[:, 0:wb8m_w])
        for i in range(1, NSL):
            stream(i)
        v2p_t = load(v2p_d)
        s2b_t = load(s2b_d)

        # unpack views
        wT_v = wTw0_t[:, 0:E]
        w0_v = wTw0_t[:, E:E + 64]
        bg1_v = m128_t[:, 0:G]
        bcol_v = m128_t[:, G:G + 1]
        ident_v = m128_t[:, G + 1:]
        NH = NBA // 2
        fA = [bigA0_t[:].rearrange("p (b e) -> p b e", b=NH),
              bigA1_t[:, 0:NH * E].rearrange("p (b e) -> p b e", b=NH)]
        s1t_v = bigA1_t[:, NH * E:(NH + 1) * E]
        ba_v = bigA1_t[:, (NH + 1) * E:(NH + 1) * E + 128]
        bb_v = bigA1_t[:, (NH + 1) * E + 128:(NH + 1) * E + 192]
        fCD = [t[:].rearrange("p (b e) -> p b e", b=2 * NJC) for t in finCD_t]
        vs3 = m64_t[:, 0:3 * E].rearrange("p (b e) -> p b e", b=3)
        d2t_v = m64_t[:, 3 * E:4 * E]
        bc_v = m64_t[:, 4 * E:4 * E + 64]
        bd_v = m64_t[:, 4 * E + 64:4 * E + 192]
        v2p = v2p_t[:].rearrange("p (b e) -> p b e", b=3)

        GS = G // NSL  # chunks per slice

        def wview(ts, g, jdim, tdim, w):
            return ts[g // GS][:].rearrange(
                "p (g j t w) -> p g j t w", g=GS, j=jdim, t=tdim)[:, g % GS]

        # MLP layer 1: h1 = relu(W0.T @ wT + b0) : [64, E]
        ps_h1 = ps_rot.tile([64, E], F32, tag="rot")
        nc.tensor.matmul(ps_h1[:], w0_v, wT_v, start=True, stop=True)
        h1_t = work.tile([64, E], BF16)
        nc.scalar.activation(
            h1_t[:], ps_h1[:], mybir.ActivationFunctionType.Relu,
            bias=0.0, scale=1.0,
        )

        # persistent PSUM accumulators
        psA = ps_acc.tile([128, E], F32, tag="A")
        psB_t = ps_acc.tile([64, E], F32, tag="B")
        psC_t = [ps_acc.tile([64, E], F32, tag=f"C{i}", name=f"psC{i}")
                 for i in range(3)]
        psD = ps_acc.tile([128, E], F32, tag="D")
        psB = psB_t[:]
        psC = [t[:] for t in psC_t]

        # b2 contributions open each accumulation group (fills head idle)
        nc.tensor.matmul(psA[:], ba_v, s1t_v, start=True, stop=False,
                         skip_group_check=True)
        nc.tensor.matmul(psB, bb_v, s1t_v, start=True, stop=False,
                         skip_group_check=True)
        for i in range(3):
            nc.tensor.matmul(psC[i], bc_v, vs3[:, i, :], start=True,
                             stop=False, skip_group_check=True)
        nc.tensor.matmul(psD[:], bd_v, d2t_v, start=True, stop=False,
                         skip_group_check=True)

        def bcast(g):
            ps_bc = ps_rot.tile([128, E], F32, tag="rot", name=f"bc{g}")
            nc.tensor.matmul(ps_bc[:], wg1_t[:, bass.ts(g, 128)], h1_t[:],
                             start=True, stop=True, skip_group_check=True)
            return ps_bc

        DR = mybir.MatmulPerfMode.DoubleRow
        pt8s = [None] * G   # retained fp8 product tiles (deferred residuals)
        ptcs = [None] * G   # retained bf16 product tiles (lagged C/D)

        def emit_res(g, stop):
            p2 = pt8s[g][:].rearrange("p (b e) -> p b e", b=NBA)
            wa = wview(wa8r_ts, g, NJA, 2, 128)
            wb = wview(wb8r_ts, g, NJA, 2, 64)
            for jp in range(NJA):
                nc.tensor.matmul(psA[:], wa[:, jp], p2[:, 2 * jp:2 * jp + 2, :],
                                 start=False, stop=stop and jp == NJA - 1,
                                 perf_mode=DR, skip_group_check=True)
            for jp in range(NJA):
                nc.tensor.matmul(psB, wb[:, jp], p2[:, 2 * jp:2 * jp + 2, :],
                                 start=False, stop=stop and jp == NJA - 1,
                                 perf_mode=DR, skip_group_check=True)

        def emit_cd0(g, last):
            pc0 = ptcs[g][0][:].rearrange("p (b e) -> p b e", b=2 * NJC)
            wcv = wview(wc_ts, g, NJC, 1, 64)
            for i in range(2):
                for j in range(NJC):
                    nc.tensor.matmul(psC[i], wcv[:, j], pc0[:, NJC * i + j, :],
                                     start=False,
                                     stop=last and j == NJC - 1,
                                     skip_group_check=True)

        def emit_cd1(g, last):
            pc1 = ptcs[g][1][:].rearrange("p (b e) -> p b e", b=2 * NJC)
            wcv = wview(wc_ts, g, NJC, 1, 64)
            wdv = wview(wd_ts, g, NJC, 1, 128)
            for j in range(NJC):
                nc.tensor.matmul(psD[:], wdv[:, j], pc1[:, NJC + j, :],
                                 start=False,
                                 stop=last and j == NJC - 1,
                                 skip_group_check=True)
            for j in range(NJC):
                nc.tensor.matmul(psC[2], wcv[:, j], pc1[:, j, :],
                                 start=False,
                                 stop=last and j == NJC - 1,
                                 skip_group_check=True)

        ps_bc = bcast(0)
        for g in range(G):
            if True:
                bct = bct_pool.tile([128, E], BF16, tag="bct")
                nc.scalar.activation(
                    bct[:], ps_bc[:], mybir.ActivationFunctionType.Relu,
                    bias=bg1_v[:, g: g + 1], scale=BETA,
                )
                # products: pt8 (A/B on Pool, fp8) ; ptc halves (C/D, bf16)
                pt8 = pt8_pool.tile([128, NBA * E], FP8, tag="pt8")
                pt8s[g] = pt8
                bct_b = dataclasses.replace(
                    bct[:], ap=[bct[:].ap[0], [0, NBA // 2], [1, E]]
                )
                p2w = pt8[:].rearrange("p (b e) -> p b e", b=NBA)
                ab0_eng = nc.vector if g < 3 else nc.gpsimd
                ab0_eng.tensor_mul(p2w[:, 0:NBA // 2, :], fA[0], bct_b)
                nc.gpsimd.tensor_mul(p2w[:, NBA // 2:NBA, :], fA[1], bct_b)
                cd_eng = nc.gpsimd if g in POOL_CD else nc.vector
                bct_c = dataclasses.replace(
                    bct[:], ap=[bct[:].ap[0], [0, 2 * NJC], [1, E]]
                )
                ptcs[g] = []
                for h in range(2):
                    ptc = ptc_pool.tile([128, 2 * NJC * E], BF16, tag="ptc")
                    ptcs[g].append(ptc)
                    cd_eng.tensor_mul(
                        ptc[:].rearrange("p (b e) -> p b e", b=2 * NJC),
                        fCD[h], bct_c)
                if g + 1 < G:
                    ps_bc = bcast(g + 1)

                p2 = pt8[:].rearrange("p (b e) -> p b e", b=NBA)
                wa = wview(wa8m_ts, g, NJA, 2, 128)
                wb = wview(wb8m_ts, g, NJA, 2, 64)
                for jp in range(NJA):
                    nc.tensor.matmul(psA[:], wa[:, jp],
                                     p2[:, 2 * jp:2 * jp + 2, :],
                                     start=False, stop=False,
                                     perf_mode=DR, skip_group_check=True)
                for jp in range(NJA):
                    nc.tensor.matmul(psB, wb[:, jp],
                                     p2[:, 2 * jp:2 * jp + 2, :],
                                     start=False, stop=False,
                                     perf_mode=DR, skip_group_check=True)
            if CD_LAG0 <= g:
                emit_cd0(g - CD_LAG0, last=False)
            if g >= CD_LAG1:
                emit_cd1(g - CD_LAG1, last=False)
            # deferred fp8 residual passes (weights arrive late; order-free)
            if G // 2 <= g < G:
                emit_res(2 * (g - G // 2), stop=False)
                emit_res(2 * (g - G // 2) + 1, stop=g == G - 1)

        # drain the lagged C/D matmuls for the final chunks
        emit_cd0(G - 2, last=False)
        emit_cd1(G - 3, last=False)
        emit_cd0(G - 1, last=True)
        emit_cd1(G - 2, last=False)
        emit_cd1(G - 1, last=True)

        # epilogue in two sample-halves so DVE overlaps PE transposes + DMA:
        # out0T = psA*s2/(ab) + (I3/b)*psD + bias ; out1T_i = psB*v2_i/(ab) + psC_i/b
        out0T = work.tile([128, E], F32)
        out1T = [work.tile([64, E], F32, tag=f"o1{i}", name=f"o1{i}")
                 for i in range(3)]
        for h in range(2):
            sl = bass.ts(h, E // 2)
            tA = work.tile([128, E // 2], F32, tag=f"tA{h}", name=f"tA{h}")
            nc.vector.tensor_mul(tA[:], psA[:, sl], s2b_t[:, sl])
            tD = work.tile([128, E // 2], F32, tag=f"tD{h}", name=f"tD{h}")
            nc.scalar.mul(tD[:], psD[:, sl], I3 / BETA)
            nc.vector.affine_then_add(out0T[:, sl], tA[:], tD[:], scale=1.0,
                                      bias=bcol_v)
            for i in range(3):
                tB = work.tile([64, E // 2], F32, tag=f"tB{i}{h}",
                               name=f"tB{i}{h}")
                nc.vector.tensor_mul(tB[:], psB[:, sl], v2p[:, i, sl])
                nc.vector.affine_then_add(out1T[i][:, sl], psC[i][:, sl],
                                          tB[:], scale=1.0 / BETA, bias=0.0)
            # transpose this half back to [E, 320] and store
            for eh in range(2):
                et = 2 * h + eh
                slt = bass.ts(et, 128)
                outS = out_pool.tile([128, 320], F32, tag="outS")
                ps_t0 = ps_rot.tile([128, E], F32, tag="rot")
                nc.tensor.transpose(ps_t0[:, 0:128], out0T[:, slt], ident_v)
                nc.scalar.copy(outS[:, 0:128], ps_t0[:, 0:128])
                nc.sync.dma_start(outp_d[slt, 0:128], outS[:, 0:128])
                o1v = outS[:, 128:320].rearrange("p (w i) -> p i w", i=3)
                for i in range(3):
                    ps_ti = ps_rot.tile([128, E], F32, tag="rot")
                    nc.tensor.transpose(ps_ti[:, 0:64], out1T[i][:, slt],
                                        ident_v[0:64, 0:64])
                    nc.scalar.copy(o1v[:, i, :], ps_ti[:, 0:64])
                nc.sync.dma_start(outp_d[slt, 128:320], outS[:, 128:320])

    nc.compile()
    return nc


_NC = None


def _get_nc():
    global _NC
    if _NC is None:
        _NC = _build_nc()
    return _NC


def _q8(x):
    return np.clip(x, -240.0, 240.0).astype(FP8_NP)


def _prep_inputs(data_in1, data_in2, weight, W0, b0, W1, b1, W2, b2, bias):
    f32 = np.float32
    data_in1 = np.ascontiguousarray(data_in1, dtype=f32)
    data_in2 = np.ascontiguousarray(data_in2, dtype=f32)
    weight = np.ascontiguousarray(weight, dtype=f32)
    W0 = np.asarray(W0, f32); b0 = np.asarray(b0, f32)
    W1 = np.asarray(W1, f32); b1 = np.asarray(b1, f32)
    W2 = np.asarray(W2, f32); b2 = np.asarray(b2, f32)
    bias = np.asarray(bias, f32)

    s1 = data_in1[:, :MUL0]                      # [N,128]
    v1 = data_in1[:, MUL0:].reshape(N, MUL1, 3)  # [N,64,3]
    s2 = data_in2[:, 0]                          # [N]
    v2 = data_in2[:, 1:4]                        # [N,3]

    def bf(x):
        return np.ascontiguousarray(x, dtype=f32).astype(BF16_NP)

    s1t = s1.T                                   # [128,N] f32
    dot12 = np.einsum("eui,ei->eu", v1, v2).T    # [64,N]
    vs = [(v1[:, :, i] * s2[:, None]).T for i in range(3)]  # [64,N] each

    def tiles_of(x, nblk):
        return [np.tile(x[TPB * j: TPB * (j + 1)], (KPC, 1))
                for j in range(nblk)]

    finA = np.stack(tiles_of(s1t, NBA), axis=1)          # [128, NBA, N]
    blk0 = tiles_of(vs[0], NJC) + tiles_of(vs[1], NJC)   # C0 | C1
    blk1 = tiles_of(vs[2], NJC) + tiles_of(dot12, NJC)   # C2 | D
    finCD0 = np.stack(blk0, axis=1)                      # [128, 8, N]
    finCD1 = np.stack(blk1, axis=1)

    wT = bf(weight.T)
    s2b = np.ascontiguousarray(
        np.broadcast_to(s2 * AB_INV, (128, N)), dtype=f32)

    # stationary chunk layouts: row r=(koff*TPB+uu) -> W[KPC*g+koff, TPB*j+uu, :]
    def chunks(arr3):  # [64,U,W] -> [128(r), G, U//TPB(j), W]
        U, W = arr3.shape[1], arr3.shape[2]
        t = arr3.reshape(G, KPC, U // TPB, TPB, W)     # [g,koff,j,uu,w]
        return np.transpose(t, (1, 3, 0, 2, 4)).reshape(128, G, U // TPB, W)

    Wa3 = W2[:, :N1].reshape(64, 128, 128)
    Wb3 = W2[:, N1:N1 + N2].reshape(64, 128, 64)
    Wc3 = W2[:, N1 + N2:N1 + N2 + N3].reshape(64, 64, 64)
    Wd3 = W2[:, N1 + N2 + N3:].reshape(64, 64, 128)

    # fp8 main + same-scale residual for A/B
    def fp8_pair(arr3):
        c = chunks(arr3) * ALPHA                       # [128, G, U//TPB, W]
        m = _q8(c)
        r = _q8(c - m.astype(f32))
        return (np.ascontiguousarray(m).reshape(128, -1),
                np.ascontiguousarray(r).reshape(128, -1))

    wa8m, wa8r = fp8_pair(Wa3)
    wb8m, wb8r = fp8_pair(Wb3)

    bg1 = (b1.reshape(G, KPC)[None, :, :].repeat(TPB, axis=0)
           .transpose(2, 0, 1).reshape(128, G) * BETA)
    m128 = np.concatenate(
        [bg1, bias.reshape(128, 1), np.eye(128, dtype=f32)], axis=1
    ).astype(f32)
    m64_shared = [bf(b2[N1 + N2:N1 + N2 + N3].reshape(64, 64) * BETA),
                  bf(b2[N1 + N2 + N3:].reshape(64, 128) * BETA)]
    ba = bf(b2[:N1].reshape(128, 128) * (ALPHA * BETA))
    bb = bf(b2[N1:N1 + N2].reshape(128, 64) * (ALPHA * BETA))

    shared = {
        "wg1": bf(np.repeat(W1, TPB, axis=1)),
        "m128": np.ascontiguousarray(m128),
        "wa8m": wa8m, "wa8r": wa8r, "wb8m": wb8m, "wb8r": wb8r,
        "wc": bf(chunks(Wc3).reshape(128, -1)),
        "wd": bf(chunks(Wd3).reshape(128, -1)),
    }
    w0b = bf(W0)

    in_maps = []
    for c in range(N_CORES):
        e0 = c * E
        sl = slice(e0, e0 + E)
        m = dict(shared)
        m["wTw0"] = np.ascontiguousarray(np.concatenate(
            [np.concatenate([wT[:, sl], np.ones((1, E), BF16_NP)], axis=0),
             np.concatenate([w0b, bf(b0).reshape(1, 64)], axis=0)], axis=1))
        NH = NBA // 2
        m["bigA0"] = np.ascontiguousarray(
            bf(finA[:, 0:NH, sl]).reshape(128, NH * E))
        m["bigA1"] = np.ascontiguousarray(np.concatenate(
            [bf(finA[:, NH:NBA, sl]).reshape(128, NH * E), bf(s1t[:, sl]),
             ba, bb], axis=1))
        m["finCD0"] = bf(finCD0[:, :, sl]).reshape(128, 2 * NJC * E)
        m["finCD1"] = bf(finCD1[:, :, sl]).reshape(128, 2 * NJC * E)
        m["m64"] = np.ascontiguousarray(np.concatenate(
            [bf(np.stack([v[:, sl] for v in vs], axis=1)).reshape(64, 3 * E),
             bf(dot12[:, sl])] + m64_shared, axis=1))
        m["v2p"] = np.ascontiguousarray(np.stack(
            [np.broadcast_to(v2[:, i] * AB_INV, (64, N))[:, sl]
             for i in range(3)], axis=1).reshape(64, 3 * E), f32)
        m["s2b"] = np.ascontiguousarray(s2b[:, sl])
        in_maps.append(m)
    return in_maps


def run(in_maps, **kwargs):
    nc = _get_nc()
    return run_bass_kernel_spmd(nc, in_maps, list(range(N_CORES)), **kwargs)


def kernel(data_in1, data_in2, weight, W0, b0, W1, b1, W2, b2, bias):
    in_maps = _prep_inputs(
        data_in1, data_in2, weight, W0, b0, W1, b1, W2, b2, bias
    )
    res = run(in_maps)
    out = np.concatenate(
        [np.asarray(res.results[c]["outp"]) for c in range(N_CORES)], axis=0
    )
    return out.astype(np.float32)
